# revision 5
# baseline (speedup 1.0000x reference)
"""Trainium2 Bass kernel for nn_MultiHeadDecoder (B=16, N=500, D=128, H=8, KD=16).

Strategy: data-parallel over batch (2 batches per core x 8 cores). Per batch,
the whole pipeline (refined embeddings -> dual multi-head compat scores ->
per-pair MLP 16->32->32->1) is fused on-chip; only inputs and the final
[N, N] output touch HBM.

Layout trick: score matmuls emit interleaved rows (i8, c16) per PSUM tile
(8 query nodes x 16 channels on the 128 partitions) so that every MLP layer
is a single full-width [128x128] stationary matmul with a host-precomputed
scatter of the layer weights. The data-dependent score lhsT (Q columns
replicated 16x and channel-masked) is built on the DVE with one bf16
broadcast-AP multiply per 32 query rows.
"""

import numpy as np
import ml_dtypes

import concourse.bacc as bacc
import concourse.mybir as mybir
import concourse.tile as tile
from concourse.bass_utils import run_bass_kernel_spmd

B, N, D, H, KD = 16, 500, 128, 8, 16
NORM = 1.0 / np.sqrt(KD)
NCORES = 8
BPC = B // NCORES          # batches per core
NP = 512                   # padded i dimension
NBLK = NP // 128           # 4 i-blocks of 128
NCHUNK = 16                # 8-i chunks per block
F32 = mybir.dt.float32
BF16 = mybir.dt.bfloat16
AF = mybir.ActivationFunctionType
ALU = mybir.AluOpType

_cache = {}


def _build_nc():
    nc = bacc.Bacc("TRN2", target_bir_lowering=False, debug=False,
                   num_devices=NCORES)

    hx = nc.declare_dram_parameter("hx", [BPC, N, D], F32, isOutput=False)
    px = nc.declare_dram_parameter("px", [BPC, N, D], F32, isOutput=False)
    wq = nc.declare_dram_parameter("wqc", [2, 128, 128], F32, isOutput=False)
    wk = nc.declare_dram_parameter("wkc", [2, 128, 128], F32, isOutput=False)
    pm = nc.declare_dram_parameter("pmats", [4, 128, 128], F32, isOutput=False)
    idm = nc.declare_dram_parameter("ident", [128, 128], F32, isOutput=False)
    l1 = nc.declare_dram_parameter("l1c", [2, 128, 128], F32, isOutput=False)
    l2 = nc.declare_dram_parameter("l2c", [128, 128], F32, isOutput=False)
    l3 = nc.declare_dram_parameter("l3c", [32, 128, 128], F32, isOutput=False)
    mrep = nc.declare_dram_parameter("mrep", [2, 128, 512], BF16, isOutput=False)
    bts = nc.declare_dram_parameter("bts", [3, 128, 1], F32, isOutput=False)
    y = nc.declare_dram_parameter("y", [BPC, N, N], F32, isOutput=True)

    with tile.TileContext(nc) as tc:
        with (
            tc.tile_pool(name="cpool", bufs=1) as cpool,
            tc.tile_pool(name="embed", bufs=2) as epool,
            tc.tile_pool(name="xnat", bufs=2) as xpool,
            tc.tile_pool(name="qk", bufs=2) as qkpool,
            tc.tile_pool(name="bld", bufs=2) as bpool,
            tc.tile_pool(name="spool", bufs=3) as spool,
            tc.tile_pool(name="h1p", bufs=3) as h1pool,
            tc.tile_pool(name="h2p", bufs=3) as h2pool,
            tc.tile_pool(name="outp", bufs=2) as opool,
            tc.tile_pool(name="ps_s", bufs=2, space="PSUM") as ps_s,
            tc.tile_pool(name="ps_h1", bufs=2, space="PSUM") as ps_h1,
            tc.tile_pool(name="ps_h2", bufs=2, space="PSUM") as ps_h2,
            tc.tile_pool(name="ps_out", bufs=1, space="PSUM") as ps_out,
            tc.tile_pool(name="ps_misc", bufs=1, space="PSUM") as ps_misc,
        ):
            # ---- constants live [128, k*w] with sub-matrix j at cols j*w ----
            wq_t = cpool.tile([128, 2 * 128], F32, tag="wq")
            wk_t = cpool.tile([128, 2 * 128], F32, tag="wk")
            pm_t = cpool.tile([128, 4 * 128], F32, tag="pm")
            id_t = cpool.tile([128, 128], F32, tag="id")
            l1_t = cpool.tile([128, 2 * 128], F32, tag="l1")
            l2_t = cpool.tile([128, 128], F32, tag="l2")
            l3_t = cpool.tile([128, 32 * 128], F32, tag="l3")
            mr_t = cpool.tile([128, 2 * 512], BF16, tag="mr")
            bt_t = cpool.tile([128, 3], F32, tag="bt")
            for dst, src, k, w in ((wq_t, wq, 2, 128), (wk_t, wk, 2, 128),
                                   (pm_t, pm, 4, 128), (l1_t, l1, 2, 128),
                                   (l3_t, l3, 32, 128), (bt_t, bts, 3, 1),
                                   (mr_t, mrep, 2, 512)):
                for j in range(k):
                    nc.sync.dma_start(dst[:, w * j:w * (j + 1)], src[j])
            nc.sync.dma_start(id_t[:], idm[:])
            nc.sync.dma_start(l2_t[:], l2[:])

            def wqs(bi):
                return wq_t[:, 128 * bi:128 * (bi + 1)]

            def wks(bi):
                return wk_t[:, 128 * bi:128 * (bi + 1)]

            def pms(j):
                return pm_t[:, 128 * j:128 * (j + 1)]

            def l1s(h):
                return l1_t[:, 128 * h:128 * (h + 1)]

            def l3s(t):
                return l3_t[:, 128 * t:128 * (t + 1)]

            def mrs(bi):
                return mr_t[:, 512 * bi:512 * (bi + 1)]

            def bias(j):
                return bt_t[:, j:j + 1]

            copy_tick = 0

            def relu_copy(dst, src, bias_ap):
                nonlocal copy_tick
                if copy_tick % 2 == 0:
                    nc.scalar.activation(dst, src, AF.Relu, bias=bias_ap)
                else:
                    nc.vector.tensor_scalar(dst, src, bias_ap, 0.0,
                                            ALU.add, ALU.max)
                copy_tick += 1

            def plain_copy(dst, src):
                nonlocal copy_tick
                if copy_tick % 2 == 0:
                    nc.scalar.copy(dst, src)
                else:
                    nc.vector.tensor_copy(dst, src)
                copy_tick += 1

            for b in range(BPC):
                # ================= embeddings =================
                # transpose x [500,128] -> xT [128,500] via PE (lhsT=x, rhs=I)
                xT = {}
                for name, src in (("h", hx), ("p", px)):
                    xt = epool.tile([128, N], F32, tag=f"xT_{name}")
                    for s in range(4):
                        rows = 125
                        xn = xpool.tile([128, 128], F32, tag="xn")
                        nc.sync.dma_start(xn[0:rows, :],
                                          src[b, rows * s:rows * (s + 1), :])
                        pt = ps_misc.tile([128, 512], F32, tag="misc")
                        nc.tensor.matmul(pt[:, 0:rows], xn[0:rows, :],
                                         id_t[0:rows, 0:rows],
                                         start=True, stop=True)
                        plain_copy(xt[:, rows * s:rows * (s + 1)],
                                   pt[:, 0:rows])
                    xT[name] = xt

                # refined embeddings h^T = P-proj + broadcast graph term
                hT = {}
                # pmats order: [P_node_node, P_graph_node, P_node_pos, P_graph_pos]
                for name, pn, pg in (("node", 0, 1), ("pos", 2, 3)):
                    src = xT["h"] if name == "node" else xT["p"]
                    xm = epool.tile([128, 1], F32, tag=f"xm_{name}")
                    nc.vector.reduce_max(xm[:], src[:, 0:N],
                                         mybir.AxisListType.X)
                    pt = ps_misc.tile([128, 512], F32, tag="misc")
                    nc.tensor.matmul(pt[:, 0:N], pms(pn), src[:],
                                     start=True, stop=True)
                    nc.tensor.matmul(pt[:, N:N + 1], pms(pg), xm[:],
                                     start=True, stop=True)
                    g_sb = epool.tile([128, 1], F32, tag=f"g_{name}")
                    nc.vector.tensor_copy(g_sb[:], pt[:, N:N + 1])
                    ht = epool.tile([128, N], F32, tag=f"hT_{name}")
                    nc.vector.tensor_scalar(ht[:], pt[:, 0:N], g_sb[:], None,
                                            ALU.add)
                    hT[name] = ht

                # ================= Q/K projections (bf16) =================
                qT, kT = {}, {}
                for bi, name in ((0, "pos"), (1, "node")):
                    qt = qkpool.tile([128, NP], BF16, tag=f"qT_{name}")
                    kt = qkpool.tile([128, N], BF16, tag=f"kT_{name}")
                    pt = ps_misc.tile([128, 512], F32, tag="misc")
                    nc.tensor.matmul(pt[:, 0:N], wqs(bi), hT[name][:],
                                     start=True, stop=True)
                    plain_copy(qt[:, 0:N], pt[:, 0:N])
                    nc.vector.memset(qt[:, N:NP], 0.0)
                    pt2 = ps_misc.tile([128, 512], F32, tag="misc")
                    nc.tensor.matmul(pt2[:, 0:N], wks(bi), hT[name][:],
                                     start=True, stop=True)
                    plain_copy(kt[:], pt2[:, 0:N])
                    qT[name] = qt
                    kT[name] = kt

                # ================= main pair loop =================
                for blk in range(NBLK):
                    po = ps_out.tile([128, N], F32, tag="out")
                    builds = [None, None]
                    for u in range(NCHUNK):
                        if u % 4 == 0:
                            g0 = blk * 128 + u * 8  # first i of this group
                            for bi, name in ((0, "pos"), (1, "node")):
                                bt = bpool.tile([128, 512], BF16,
                                                tag=f"bld{bi}")
                                src_ap = (qT[name][:, g0:g0 + 32]
                                          .unsqueeze(2)
                                          .broadcast_to([128, 32, 16]))
                                msk_ap = mrs(bi).rearrange(
                                    "p (i c) -> p i c", i=32)
                                nc.vector.tensor_tensor(
                                    bt[:].rearrange("p (i c) -> p i c", i=32),
                                    src_ap, msk_ap, ALU.mult)
                                builds[bi] = bt
                        uu = u % 4
                        ps = ps_s.tile([128, N], F32, tag="s")
                        nc.tensor.matmul(ps[:],
                                         builds[0][:, 128 * uu:128 * (uu + 1)],
                                         kT["pos"][:], start=True, stop=False)
                        nc.tensor.matmul(ps[:],
                                         builds[1][:, 128 * uu:128 * (uu + 1)],
                                         kT["node"][:], start=False, stop=True)
                        s_sb = spool.tile([128, N], F32, tag="s_sb")
                        plain_copy(s_sb[:], ps[:])
                        for half in range(2):
                            ph1 = ps_h1.tile([128, N], F32, tag="h1")
                            nc.tensor.matmul(ph1[:], l1s(half), s_sb[:],
                                             start=True, stop=True)
                            h1_sb = h1pool.tile([128, N], F32, tag="h1_sb")
                            relu_copy(h1_sb[:], ph1[:], bias(0))
                            ph2 = ps_h2.tile([128, N], F32, tag="h2")
                            nc.tensor.matmul(ph2[:], l2_t[:], h1_sb[:],
                                             start=True, stop=True)
                            h2_sb = h2pool.tile([128, N], F32, tag="h2_sb")
                            relu_copy(h2_sb[:], ph2[:], bias(1))
                            t = 2 * u + half
                            nc.tensor.matmul(po[:], l3s(t), h2_sb[:],
                                             start=(t == 0), stop=(t == 31),
                                             skip_group_check=True)
                    out_sb = opool.tile([128, N], F32, tag="out_sb")
                    nc.vector.tensor_scalar(out_sb[:], po[:], bias(2),
                                            None, ALU.add)
                    rows = min(128, N - 128 * blk)
                    nc.sync.dma_start(y[b, 128 * blk:128 * blk + rows, :],
                                      out_sb[0:rows, :])

    nc.compile()
    return nc


def _consts(Wq_pos, Wk_pos, Wq_node, Wk_node,
            P_graph_pos, P_graph_node, P_node_pos, P_node_node,
            W1, b1, W2, b2, W3, b3):
    f32 = np.float32

    # W [H, D, KD] -> cat [D, H*KD], col c*16+k = W[c, :, k]
    def cat(w, scale=1.0):
        return (np.transpose(w, (1, 0, 2)).reshape(D, H * KD) * scale).astype(f32)

    wqc = np.stack([cat(Wq_pos, NORM), cat(Wq_node, NORM)])
    wkc = np.stack([cat(Wk_pos), cat(Wk_node)])
    pmats = np.stack([P_node_node, P_graph_node,
                      P_node_pos, P_graph_pos]).astype(f32)
    ident = np.eye(128, dtype=f32)

    # L1 scatter: rows (i8, c16) -> cols (i4, m32); half selects i 0-3 / 4-7
    l1c = np.zeros((2, 128, 128), f32)
    for half in range(2):
        for i in range(8):
            ip = i - 4 * half
            if 0 <= ip < 4:
                for c in range(16):
                    l1c[half, 16 * i + c, 32 * ip:32 * ip + 32] = W1[c]

    l2c = np.zeros((128, 128), f32)
    for g in range(4):
        l2c[32 * g:32 * g + 32, 32 * g:32 * g + 32] = W2

    l3c = np.zeros((32, 128, 128), f32)
    for t in range(32):
        for q in range(4):
            l3c[t, 32 * q:32 * q + 32, 4 * t + q] = W3[:, 0]

    # channel masks: branch 0 (pos) covers channels 0-7, branch 1 -> 8-15
    mrep = np.zeros((2, 128, 512), np.float32)
    for bi in range(2):
        for cp in range(8):
            mrep[bi, 16 * cp:16 * cp + 16,
                 (8 * bi + cp)::16] = 1.0
    mrep = mrep.astype(ml_dtypes.bfloat16)

    bts = np.stack([
        np.tile(np.asarray(b1, f32), 4)[:, None],
        np.tile(np.asarray(b2, f32), 4)[:, None],
        np.full((128, 1), np.asarray(b3, f32).reshape(-1)[0], f32),
    ])
    return dict(wqc=wqc, wkc=wkc, pmats=pmats, ident=ident, l1c=l1c,
                l2c=l2c, l3c=l3c, mrep=mrep, bts=bts)


def kernel(h_em, pos_em, solving_state_for_net,
           Wq_pos, Wk_pos, Wq_node, Wk_node,
           P_graph_pos, P_graph_node, P_node_pos, P_node_node,
           W1, b1, W2, b2, W3, b3):
    if "nc" not in _cache:
        _cache["nc"] = _build_nc()
    nc = _cache["nc"]

    consts = _consts(np.asarray(Wq_pos), np.asarray(Wk_pos),
                     np.asarray(Wq_node), np.asarray(Wk_node),
                     np.asarray(P_graph_pos), np.asarray(P_graph_node),
                     np.asarray(P_node_pos), np.asarray(P_node_node),
                     np.asarray(W1), np.asarray(b1), np.asarray(W2),
                     np.asarray(b2), np.asarray(W3), np.asarray(b3))

    h_em = np.asarray(h_em, np.float32)
    pos_em = np.asarray(pos_em, np.float32)
    in_maps = []
    for c in range(NCORES):
        m = dict(consts)
        m["hx"] = h_em[BPC * c:BPC * (c + 1)]
        m["px"] = pos_em[BPC * c:BPC * (c + 1)]
        in_maps.append(m)

    import os
    trace = bool(os.environ.get("KTRACE"))
    res = run_bass_kernel_spmd(nc, in_maps, list(range(NCORES)), trace=trace)
    _cache["last_res"] = res
    out = np.concatenate([r["y"] for r in res.results], axis=0)
    return out.astype(np.float32)
